# revision 24
# baseline (speedup 1.0000x reference)
"""Trainium2 Bass kernel for nn_AttentionBlock (GroupNorm + MHA + proj + residual).

Sharding: data-parallel over batch; 8 batches -> 8 NeuronCores, one batch each.

v2 design (vs baseline at 181us):
  - bf16 on the whole matmul path (x, weights, h, q, k, es, vTa, at); GN stats,
    psum accumulation, softmax Z and the residual stay f32.  Halves input DMA.
  - input DMA spread over 4 engine queues, x first; consts packed into one DMA.
  - warmup matmuls on a scratch tile bridge the HAM activity window so real
    matmuls run at 2.4 GHz from the start.
  - score matmuls K=64 per head, packed as concurrent PE row-tiles
    (head even on partitions 0-63, head odd on 64-127).
  - softmax denominator via ones-column in vTa (M=65 AV matmuls); normalization
    via DVE reciprocal of the psum Z row + gpsimd partition_broadcast + one
    fused DVE multiply.  No DRAM round trips.
  - output stores split per 512-col half across all 4 queues.
"""

import numpy as np

B, C, HW, T = 8, 512, 32, 1024
H, CH = 8, 64
G, GS = 32, 16
EPS = 1e-5
NCORES = 8

WARM_N = 28       # warmup matmuls (N=512) to hold the HAM window until real work
USE_PB = False    # gpsimd partition_broadcast for 1/Z (unsupported by this
                  # walrus: "ISA wrong length"); else PE broadcast

_CACHE = {}
TRACE = False  # test harness can set kernel.TRACE = True to get a profile


def _install_ntff_hook():
    import sys, types
    if 'antenv.axon_hooks' in sys.modules:
        return
    mod = types.ModuleType('antenv.axon_hooks')
    state = {'hook': None}
    mod.set_axon_ntff_profile_hook = lambda h: state.__setitem__('hook', h)
    mod.get_axon_ntff_profile_hook = lambda: state['hook']
    sys.modules['antenv.axon_hooks'] = mod
    import antenv
    antenv.axon_hooks = mod
    try:
        from trn_agent_boot.trn_boot import _ntff_profile_via_ctypes
        mod.set_axon_ntff_profile_hook(_ntff_profile_via_ctypes('/opt/axon/libaxon_pjrt.so'))
    except Exception:
        pass


def _patch_ldw_opt():
    """Let walrus dedup back-to-back LDWEIGHTS of the same stationary operand."""
    import concourse.bass_utils as bu
    if getattr(bu, "_ldw_opt_patched", False):
        return
    orig = bu.run_command

    def patched(argv, **kw):
        argv = ["--enable-ldw-opt=true" if a == "--enable-ldw-opt=false" else a
                for a in argv]
        return orig(argv, **kw)

    bu.run_command = patched
    bu._ldw_opt_patched = True


def _split_multi_waits(nc, max_waits=1):
    """This container's walrus supports only one sync wait per instruction; move
    extra waits onto same-engine no-ops inserted just before the instruction."""
    import concourse.mybir as mybir
    for f in nc.m.functions:
        for bb in f.blocks:
            insts = bb.instructions
            out = []
            changed = False
            for inst in insts:
                si = inst.sync_info
                waits = list(si.on_wait) if si is not None and si.on_wait else []
                if len(waits) > max_waits:
                    changed = True
                    for j, w in enumerate(waits[:-max_waits]):
                        out.append(mybir.InstNoOp(
                            name=f"{inst.name}-ws{j}",
                            sync_info=mybir.SyncInfo(on_wait=[w], on_update=[]),
                            bass_nofuse=True,
                            engine=inst.engine,
                        ))
                    inst.sync_info = mybir.SyncInfo(
                        on_wait=waits[-max_waits:],
                        on_update=list(si.on_update) if si.on_update else [],
                    )
                out.append(inst)
            if changed:
                bb.instructions = out


def _build_nc():
    import concourse.bass as bass
    import concourse.tile as tile
    import concourse.mybir as mybir

    f32 = mybir.dt.float32
    f32r = mybir.dt.float32r
    bf16 = mybir.dt.bfloat16
    Alu = mybir.AluOpType
    Act = mybir.ActivationFunctionType

    nc = bass.Bass()

    # x packed host-side as [p, (ci, t)] so each partition reads 4KB+
    # contiguous (2KB descriptors halve the effective DMA rate)
    xin = nc.dram_tensor("xin", [128, 4096], bf16, kind="ExternalInput")
    # q|k weights packed by pair-group: [g, c-part, (pair, ci, q128|k128)]
    wqkP = nc.dram_tensor("wqkP", [2, 128, 2048], bf16, kind="ExternalInput")
    wvT = nc.dram_tensor("wvT", [C, C], bf16, kind="ExternalInput")
    wpT = nc.dram_tensor("wpT", [C, C], bf16, kind="ExternalInput")
    # packed small consts: cols 0:4 gam, 4:8 bet, 8:16 bqk(by oi), 16:20 bp
    cpack = nc.dram_tensor("cpack", [128, 20], f32, kind="ExternalInput")
    maskA = nc.dram_tensor("maskA", [128, 8], f32r, kind="ExternalInput")
    maskB = nc.dram_tensor("maskB", [8, 128], f32r, kind="ExternalInput")
    # h only, [p, (ci, t)] packed bf16; residual x + unpack happen host-side
    outd = nc.dram_tensor("outd", [128, 4096], bf16, kind="ExternalOutput")

    store_engs = []  # round-robin for output stores

    with tile.TileContext(nc) as tc:
        with tc.tile_pool(name="const", bufs=1) as const, \
             tc.tile_pool(name="big", bufs=1) as big, \
             tc.tile_pool(name="qpp", bufs=2) as qpp, \
             tc.tile_pool(name="kpp", bufs=2) as kpp, \
             tc.tile_pool(name="esp", bufs=16) as esp, \
             tc.tile_pool(name="accp", bufs=1) as accp, \
             tc.tile_pool(name="zp", bufs=2) as zp, \
             tc.tile_pool(name="gn", bufs=2) as gn, \
             tc.tile_pool(name="ps", bufs=2, space="PSUM") as ps, \
             tc.tile_pool(name="dram", bufs=2, space="DRAM") as dram:

            # ---- warmup scaffolding; actual warm matmuls are emitted in
            # phases below (plain first, then gated on x arrivals) so the PE
            # stays busy through the DMA/GroupNorm window without blocking
            # real work.  warm memset on gpsimd so the tensor queue can start
            # ~2us before the DVE preamble finishes. ----
            warm = const.tile([128, 512], bf16)
            nc.gpsimd.memset(warm.bitcast(f32), 0.0)
            wi = [0]

            def warms(n, src=None):
                src = warm if src is None else src
                for _ in range(n):
                    pw = ps.tile([128, 512], f32, tag="s", name=f"warm{wi[0]}")
                    nc.tensor.matmul(pw, lhsT=src[:, 0:128],
                                     rhs=src[:, 0:512], start=True, stop=True)
                    wi[0] += 1

            # one contiguous burst > the 3.4us HAM window so the clock gate
            # actually opens; the x-gated batches below then keep it open.
            warms(12)
            onesr = const.tile([1, 64], f32r)
            nc.vector.memset(onesr.bitcast(f32), 1.0)
            ones1p = const.tile([1, 128], f32r)
            nc.vector.memset(ones1p.bitcast(f32), 1.0)
            onesf = const.tile([128, 64], f32)
            nc.vector.memset(onesf, 1.0)

            # ---- input DMA.  Only sync/scalar/gpsimd queues can issue DMAs;
            # the gpsimd (software-DGE) queue is ~4x slower, so it only gets
            # wpT (needed last).  x first in 4 chunks so GroupNorm stats can
            # start per-chunk; weights strictly behind x on both fast rings.
            xt2 = [big.tile([128, 2, 1024], bf16, tag=f"x{g}", name=f"xg{g}")
                   for g in range(2)]
            xtv = [xt2[ci // 2][:, ci % 2, :] for ci in range(4)]
            # tiny consts on the (otherwise idle) gpsimd ring so they don't
            # delay the scalar ring's x chunk
            cp_t = const.tile([128, 20], f32)
            nc.gpsimd.dma_start(out=cp_t, in_=cpack[:, :])
            mA = const.tile([128, 8], f32r)
            nc.gpsimd.dma_start(out=mA, in_=maskA[:, :])
            mB = const.tile([8, 128], f32r)
            nc.gpsimd.dma_start(out=mB, in_=maskB[:, :])
            xinr = xin.rearrange("p (ci t) -> p ci t", t=1024)
            nc.sync.dma_start(out=xt2[0][:, 0, :], in_=xinr[:, 0, :])
            nc.sync.dma_start(out=xt2[0][:, 1, :], in_=xinr[:, 1, :])
            nc.scalar.dma_start(out=xt2[1][:, 0, :], in_=xinr[:, 2, :])
            nc.scalar.dma_start(out=xt2[1][:, 1, :], in_=xinr[:, 3, :])
            gam_t = cp_t[:, 0:4]
            bet_t = cp_t[:, 4:8]
            bqk_t = cp_t[:, 8:16]
            bp_t = cp_t[:, 16:20]

            # weights behind x: qk pairs 0/1 on sync, wvT then qk pairs 2/3
            # on scalar, wpT (needed last) on the slow gpsimd queue
            wq2 = [const.tile([128, 2, 4, 256], bf16, tag=f"wqg{g}", name=f"wqg{g}")
                   for g in range(2)]
            wq_t = [wq2[pi // 2][:, pi % 2] for pi in range(4)]
            wvT_t = const.tile([128, 4, 512], bf16)
            nc.sync.dma_start(out=wq2[0], in_=wqkP[0])
            nc.scalar.dma_start(out=wvT_t, in_=wvT.rearrange("(ci p) o -> p ci o", p=128))
            nc.scalar.dma_start(out=wq2[1], in_=wqkP[1])
            wpT_t = const.tile([128, 4, 512], bf16)
            nc.gpsimd.dma_start(out=wpT_t, in_=wpT.rearrange("(ci p) o -> p ci o", p=128))

            # dummy exp after the doorbells so the ACT table set loads during
            # the DMA wait without delaying the scalar ring's x chunk
            dummy = const.tile([1, 4], f32)
            nc.scalar.activation(out=dummy, in_=warm.bitcast(f32)[0:1, 0:4],
                                 func=Act.Exp)
            def ldw_warms(n):
                # keep the PE array streaming without touching psum or the
                # tile pools: dangling weight loads only
                for _ in range(n):
                    nc.tensor.ldweights(weights=warm[:, 0:128])

            # keep the PE warm while x lands; the GN matmuls (pg, pc) are
            # interleaved below so they run as soon as their DVE inputs land
            warms(3, src=xtv[0])
            warms(3, src=xtv[1])

            # ---- GroupNorm ----
            chmv = gn.tile([128, 4, 2], f32)
            for ci in range(4):
                st = gn.tile([128, 2, 6], f32, tag="st")
                xv = xtv[ci].rearrange("p (n f) -> p n f", f=512)
                for sub in range(2):
                    nc.vector.bn_stats(out=st[:, sub, :], in_=xv[:, sub, :])
                nc.vector.bn_aggr(out=chmv[:, ci, :], in_=st)
            s2ch = gn.tile([128, 4, 2], f32r)
            nc.vector.tensor_copy(out=s2ch[:, :, 0], in_=chmv[:, :, 0])
            t1 = gn.tile([128, 4], f32)
            nc.vector.tensor_mul(out=t1, in0=chmv[:, :, 0], in1=chmv[:, :, 0])
            nc.vector.tensor_add(out=s2ch[:, :, 1], in0=t1, in1=chmv[:, :, 1])
            pg = ps.tile([128, 1024], f32, tag="s", name="pgn")
            nc.tensor.matmul(pg[0:8, 0:8], lhsT=mA,
                             rhs=s2ch.rearrange("p a b -> p (a b)"),
                             start=True, stop=True)
            # bridge the DVE Newton window so the PE clock stays up between
            # the pg and pc matmuls
            warms(7, src=xtv[2])
            warms(7, src=xtv[3])
            gf = gn.tile([8, 4, 2], f32r)
            mg = gn.tile([8, 4], f32)
            nc.vector.tensor_scalar_mul(out=mg, in0=pg[0:8, 0:8].rearrange(
                "g (a b) -> g a b", b=2)[:, :, 0], scalar1=1.0 / GS)
            vg = gn.tile([8, 4], f32)
            nc.vector.tensor_scalar_mul(out=vg, in0=pg[0:8, 0:8].rearrange(
                "g (a b) -> g a b", b=2)[:, :, 1], scalar1=1.0 / GS)
            m2 = gn.tile([8, 4], f32)
            nc.vector.tensor_mul(out=m2, in0=mg, in1=mg)
            nc.vector.tensor_sub(out=vg, in0=vg, in1=m2)
            # rstd = 1/sqrt(v+eps) via Newton on DVE (v is ~1 for unit-normal
            # x so x0 = 1.5-0.5v has rel err <6e-3; one iteration squares it
            # to <5e-5 which is far below the bf16 noise floor)
            nc.vector.tensor_scalar_add(out=vg, in0=vg, scalar1=EPS)
            yv = gn.tile([8, 4], f32)
            nc.vector.tensor_scalar(out=yv, in0=vg, scalar1=-0.5, scalar2=1.5,
                                    op0=Alu.mult, op1=Alu.add)
            for _ in range(1):
                yy = gn.tile([8, 4], f32, tag="yy")
                nc.vector.tensor_mul(out=yy, in0=yv, in1=yv)
                nc.vector.tensor_mul(out=yy, in0=yy, in1=vg)
                nc.vector.tensor_scalar(out=yy, in0=yy, scalar1=-0.5, scalar2=1.5,
                                        op0=Alu.mult, op1=Alu.add)
                nc.vector.tensor_mul(out=yv, in0=yv, in1=yy)
            nc.vector.tensor_copy(out=gf[:, :, 0], in_=mg)
            nc.vector.tensor_copy(out=gf[:, :, 1], in_=yv)
            pc = ps.tile([128, 1024], f32, tag="s", name="pgc")
            nc.tensor.matmul(pc[:, 0:8], lhsT=mB,
                             rhs=gf.rearrange("g a b -> g (a b)"),
                             start=True, stop=True)
            chms = pc[:, 0:8].rearrange("p (a b) -> p a b", b=2)
            scl = gn.tile([128, 4], f32)
            nc.vector.tensor_mul(out=scl, in0=gam_t, in1=chms[:, :, 1])
            sht = gn.tile([128, 4], f32)
            nc.vector.tensor_mul(out=sht, in0=scl, in1=chms[:, :, 0])
            nc.vector.tensor_sub(out=sht, in0=bet_t, in1=sht)
            ldw_warms(6)
            # h = x * scl + sht  (bf16; DVE for ci 0,2; ACT (idle until the
            # first softmax) for ci 1,3 so qkv isn't gated on one engine)
            ht = [big.tile([128, 1024], bf16, tag=f"h{ci}", name=f"h{ci}") for ci in range(4)]
            for ci in range(4):
                if ci % 2 == 0:
                    nc.vector.tensor_scalar(out=ht[ci], in0=xtv[ci],
                                            scalar1=scl[:, ci:ci + 1], scalar2=sht[:, ci:ci + 1],
                                            op0=Alu.mult, op1=Alu.add)
                else:
                    nc.scalar.activation(out=ht[ci], in_=xtv[ci], func=Act.Identity,
                                         scale=scl[:, ci:ci + 1], bias=sht[:, ci:ci + 1])

            # ---- vT[s, (ti, hd, ch+1)]; ones column memset once up front ----
            vTa = big.tile([128, 8, 8, 65], bf16)
            nc.vector.tensor_copy(
                out=vTa[:, :, :, 64:65],
                in_=onesf.rearrange("p (a b c) -> p a b c", a=8, b=8))

            def v_thunk(ti):
                # bv is folded into bp host-side (softmax rows sum to 1), so
                # the evacuation is a plain copy.  Per-ti granularity keeps
                # filler chunks small so the exp stream isn't starved.
                def t():
                    pv = ps.tile([128, 512], f32, tag="s", name=f"pv{ti}")
                    for ci in range(4):
                        nc.tensor.matmul(pv,
                                         lhsT=ht[ci][:, ti * 128:(ti + 1) * 128],
                                         rhs=wvT_t[:, ci, :], start=(ci == 0), stop=(ci == 3))
                    nc.vector.tensor_copy(
                        out=vTa[:, ti, :, 0:64],
                        in_=pv.rearrange("p (h c) -> p h c", h=8))
                return t

            # ---- q,k for one head pair.  qp: [q_even; q_odd] on 128 partitions,
            # kp: [k_even; k_odd] likewise (no zero padding: scores use K=64
            # row-tiles so the two heads run concurrently on the PE). ----
            qp = [None] * 4
            kp = [None] * 4

            def qkv_pair_thunks(pi):
                qp[pi] = qpp.tile([128, 1024], bf16, tag="qp", name=f"qp{pi}")
                kp[pi] = kpp.tile([128, 1024], bf16, tag="kp", name=f"kp{pi}")
                thunks = []
                state = {}

                def mk_mm(side, ci):
                    def t():
                        oi = side * 4 + pi
                        if ci == 0:
                            state[side] = ps.tile([128, 1024], f32, tag="s",
                                                  name=f"pqk{oi}")
                        pqk = state[side]
                        for ni in range(2):
                            nc.tensor.matmul(pqk[:, ni * 512:(ni + 1) * 512],
                                             lhsT=wq_t[pi][:, ci, side * 128:(side + 1) * 128],
                                             rhs=ht[ci][:, ni * 512:(ni + 1) * 512],
                                             start=(ci == 0), stop=(ci == 3))
                    return t

                def mk_evac(side):
                    def t():
                        oi = side * 4 + pi
                        dst = qp[pi] if side == 0 else kp[pi]
                        nc.vector.tensor_scalar_add(out=dst, in0=state[side],
                                                    scalar1=bqk_t[:, oi:oi + 1])
                    return t

                for side in range(2):
                    for ci in range(4):
                        thunks.append(mk_mm(side, ci))
                    thunks.append(mk_evac(side))
                return thunks

            # ---- attention core (one head pair), with PE row-tile packing on
            # the score matmuls and filler thunks to fill PE slack ----
            at_ = [None] * 4
            acc_all = accp.tile([128, 4, 1024], bf16)
            acc = [acc_all[:, oi, :] for oi in range(4)]
            pa_all = [None] * 4

            def attn_core(pi, filler=()):
                filler = list(filler)
                pa = [ps.tile([128, 1024], f32, tag="a", name=f"pa{pi}_{i}") for i in range(2)]
                pa_all[pi] = pa
                for si in range(8):
                    if si >= 1 and filler:
                        # pace fillers so none spill past the si loop (spills
                        # land on the pair transition and stall the exp
                        # stream long enough to re-throttle the PE clock)
                        n = -(-len(filler) // (8 - si))
                        for _ in range(n):
                            if filler:
                                filler.pop(0)()
                    pss = [ps.tile([128, 1024], f32, tag="s", name=f"pss{pi}_{si}_{i}")
                           for i in range(2)]
                    # concurrent row-tiles: head even rows 0-63, head odd 64-127
                    for ni in range(2):
                        for half in range(2):
                            lo, hi = half * 64, half * 64 + 64
                            nc.tensor.matmul(
                                pss[half][:, ni * 512:(ni + 1) * 512],
                                lhsT=kp[pi][lo:hi, si * 128:(si + 1) * 128],
                                rhs=qp[pi][lo:hi, ni * 512:(ni + 1) * 512],
                                start=True, stop=True)
                    ess = []
                    for half in range(2):
                        es = esp.tile([128, 1024], bf16, tag="es")
                        nc.scalar.activation(out=es, in_=pss[half], func=Act.Exp)
                        ess.append(es)
                    for half in range(2):
                        hd = 2 * pi + half
                        for ni in range(2):
                            nc.tensor.matmul(pa[half][0:65, ni * 512:(ni + 1) * 512],
                                             lhsT=vTa[:, si, hd, :],
                                             rhs=ess[half][:, ni * 512:(ni + 1) * 512],
                                             start=(si == 0), stop=(si == 7))
                for t in filler:
                    t()

            div_state = [None] * 4

            def divide_start(pi, act_copy=False):
                # Z reciprocal via [128, 8] reshape: one sbuf->sbuf reshape
                # DMA, reciprocal, then a DRAM bounce for the broadcast read.
                # Evacuate pa first so its psum banks free immediately; for
                # the last pair half 1 evacuates on the (now idle) ACT engine
                # so both evacs run concurrently.
                at_[pi] = big.tile([128, 1024], bf16, tag=f"at{pi}", name=f"at{pi}")
                pa = pa_all[pi]
                aus, zbs = [], []
                for half in range(2):
                    au = zp.tile([65, 1024], f32, tag="au")
                    if act_copy and half == 1:
                        nc.scalar.activation(out=au, in_=pa[half][0:65, :],
                                             func=Act.Copy)
                    else:
                        nc.vector.tensor_copy(out=au, in_=pa[half][0:65, :])
                    aus.append(au)
                for half in range(2):
                    eng = nc.sync if half == 0 else nc.scalar
                    zq = zp.tile([128, 8], f32, tag="zq")
                    eng.dma_start(out=zq, in_=aus[half][64:65, :].rearrange(
                        "p (a b) -> p a b", b=8))
                    nc.vector.reciprocal(out=zq, in_=zq)
                    zd2 = dram.tile([1, 1024], f32, tag="zd2")
                    eng.dma_start(
                        out=bass.AP(tensor=zd2.tensor, offset=zd2.offset,
                                    ap=[[8, 128], [1, 8]]), in_=zq)
                    zb = zp.tile([64, 1024], f32, tag="zb")
                    eng.dma_start(out=zb, in_=bass.AP(
                        tensor=zd2.tensor, offset=zd2.offset, ap=[[0, 64], [1, 1024]]))
                    zbs.append(zb)
                div_state[pi] = (aus, zbs)

            def divide_finish(pi, half):
                # emitted well after divide_start so the in-order DVE queue
                # never stalls waiting for the bounce
                def t():
                    aus, zbs = div_state[pi]
                    nc.vector.tensor_tensor(
                        out=at_[pi][half * 64:half * 64 + 64, :],
                        in0=aus[half][0:64, :], in1=zbs[half], op=Alu.mult)
                return t

            def proj_combo012(oi):
                # pi 0..2 contributions accumulate in psum; one fused evac
                # adds the bias (residual x is added host-side).  Runs as
                # phase-A fillers so only pair 3's proj piece is in the tail.
                def t():
                    pp = ps.tile([128, 1024], f32, tag="s", name=f"pp012_{oi}")
                    for pi in range(3):
                        for ni in range(2):
                            nc.tensor.matmul(pp[:, ni * 512:(ni + 1) * 512],
                                             lhsT=wpT_t[:, pi, oi * 128:(oi + 1) * 128],
                                             rhs=at_[pi][:, ni * 512:(ni + 1) * 512],
                                             start=(pi == 0), stop=(pi == 2))
                    nc.vector.tensor_scalar_add(out=acc[oi], in0=pp,
                                                scalar1=bp_t[:, oi:oi + 1])
                return t

            # ---- pair 3: ni-split attention so Z(t) for the first t-half is
            # final while the second half's AV still streams; softmax divide
            # via DVE recip on a [128,4] reshape + PE row-broadcast of the
            # reciprocal into pa's unused rows 64:127.  No DRAM round trip.
            es3 = []

            def attn3_phaseA(filler=()):
                filler = list(filler)
                pa = [ps.tile([128, 1024], f32, tag="a", name=f"pa3_{i}") for i in range(2)]
                pa_all[3] = pa
                for si in range(8):
                    if si >= 1 and filler:
                        n = -(-len(filler) // (8 - si))
                        for _ in range(n):
                            if filler:
                                filler.pop(0)()
                    pss = [ps.tile([128, 1024], f32, tag="s", name=f"pss3_{si}_{i}")
                           for i in range(2)]
                    for ni in range(2):
                        for half in range(2):
                            lo, hi = half * 64, half * 64 + 64
                            nc.tensor.matmul(
                                pss[half][:, ni * 512:(ni + 1) * 512],
                                lhsT=kp[3][lo:hi, si * 128:(si + 1) * 128],
                                rhs=qp[3][lo:hi, ni * 512:(ni + 1) * 512],
                                start=True, stop=True)
                    # ni=1 AVs lag 2 si behind: keeps the PE duty high enough
                    # through phase A that the HAM never drops the clock, and
                    # shrinks phase B to the last two si.  Emitted BEFORE this
                    # si's exp-gated AVs so they fill the exp wait.
                    if si >= 2:
                        sj = si - 2
                        for half in range(2):
                            nc.tensor.matmul(pa[half][0:65, 512:1024],
                                             lhsT=vTa[:, sj, 6 + half, :],
                                             rhs=es3[sj][half][:, 512:1024],
                                             start=(sj == 0), stop=False)
                    ess = []
                    for half in range(2):
                        es = esp.tile([128, 1024], bf16, tag="es")
                        nc.scalar.activation(out=es, in_=pss[half], func=Act.Exp)
                        ess.append(es)
                    es3.append(ess)
                    for half in range(2):
                        nc.tensor.matmul(pa[half][0:65, 0:512],
                                         lhsT=vTa[:, si, 6 + half, :],
                                         rhs=ess[half][:, 0:512],
                                         start=(si == 0), stop=(si == 7))
                for t in filler:
                    t()

            def attn3_phaseB(si_range):
                pa = pa_all[3]
                for si in si_range:
                    for half in range(2):
                        nc.tensor.matmul(pa[half][0:65, 512:1024],
                                         lhsT=vTa[:, si, 6 + half, :],
                                         rhs=es3[si][half][:, 512:1024],
                                         start=False, stop=(si == 7))

            at3 = big.tile([128, 1024], bf16, name="at3")
            au3 = [None, None]
            rz3 = {}

            def div3_evac(ni, half):
                # numerator (and the Z row in row 64) psum -> sbuf; half 1 on
                # the ACT engine which is idle after the last exp
                if au3[half] is None:
                    au3[half] = zp.tile([65, 1024], f32, tag="au",
                                        name=f"au3_{half}")
                dst = au3[half][0:65, ni * 512:(ni + 1) * 512]
                src = pa_all[3][half][0:65, ni * 512:(ni + 1) * 512]
                if half == 1:
                    nc.scalar.activation(out=dst, in_=src, func=Act.Copy)
                else:
                    nc.vector.tensor_copy(out=dst, in_=src)

            def div3_zq(ni):
                for half in range(2):
                    eng = nc.sync if half == 0 else nc.scalar
                    zq = zp.tile([128, 4], f32, tag="zq3", name=f"zq3_{ni}_{half}")
                    eng.dma_start(out=zq, in_=au3[half][64:65, ni * 512:(ni + 1) * 512]
                                  .rearrange("p (a b) -> p a b", b=4))
                    rz3[(ni, half, "zq")] = zq

            def div3_recip_rz(ni):
                for half in range(2):
                    eng = nc.sync if half == 0 else nc.scalar
                    zq = rz3[(ni, half, "zq")]
                    nc.vector.reciprocal(out=zq, in_=zq)
                    rz = zp.tile([1, 512], f32r, tag="rz3", name=f"rz3_{ni}_{half}")
                    eng.dma_start(out=rz.bitcast(f32).rearrange(
                        "p (a b) -> p a b", b=4), in_=zq)
                    rz3[(ni, half)] = rz

            def div3_bcast(ni):
                # 1/Z row -> all 128 pa rows via a K=1 ones matmul (matmul
                # dst must start at partition 0); the numerator was already
                # evacuated to au3 so the overwrite is safe
                for half in range(2):
                    nc.tensor.matmul(
                        pa_all[3][half][:, ni * 512:(ni + 1) * 512],
                        lhsT=ones1p, rhs=rz3[(ni, half)], start=True, stop=True)

            def div3_mult(ni):
                for half in range(2):
                    nc.vector.tensor_tensor(
                        out=at3[half * 64:half * 64 + 64, ni * 512:(ni + 1) * 512],
                        in0=au3[half][0:64, ni * 512:(ni + 1) * 512],
                        in1=pa_all[3][half][0:64, ni * 512:(ni + 1) * 512],
                        op=Alu.mult)

            pp3 = {}

            def proj3_mm(oi, ni, tag):
                pp = ps.tile([128, 512], f32, tag=tag, name=f"pp3_{oi}_{ni}")
                pp3[(oi, ni)] = pp
                nc.tensor.matmul(pp, lhsT=wpT_t[:, 3, oi * 128:(oi + 1) * 128],
                                 rhs=at3[:, ni * 512:(ni + 1) * 512],
                                 start=True, stop=True)

            def proj3_fin(oi, ni):
                nc.vector.tensor_add(
                    out=acc_all[:, oi, ni * 512:(ni + 1) * 512],
                    in0=acc_all[:, oi, ni * 512:(ni + 1) * 512],
                    in1=pp3[(oi, ni)])

            def store_pair(opair):
                # split by partition halves across both fast rings: full-rate
                # 4KB descriptors, half the bytes per queue
                outr = outd.rearrange("p (ci t) -> p ci t", t=1024)
                lo, hi = opair * 2, opair * 2 + 2
                nc.sync.dma_start(out=outr[0:64, lo:hi, :],
                                  in_=acc_all[0:64, lo:hi, :])
                nc.scalar.dma_start(out=outr[64:128, lo:hi, :],
                                    in_=acc_all[64:128, lo:hi, :])

            # ---- schedule ----
            q0 = qkv_pair_thunks(0)
            for t in q0:
                t()
            v_thunks = [v_thunk(ti) for ti in range(8)]
            v_thunks[0]()
            q1 = qkv_pair_thunks(1)
            # interleave so v(ti) stays ahead of AV(si)
            f0 = [v_thunks[1], v_thunks[2], v_thunks[3], q1[0],
                  v_thunks[4], q1[1], v_thunks[5], q1[2],
                  v_thunks[6], q1[3], v_thunks[7]] + q1[4:]
            attn_core(0, filler=f0)
            q2 = qkv_pair_thunks(2)
            divide_start(0)
            # transition warms: pair hand-offs idle the PE for ~2-3.4us,
            # right at the HAM re-throttle threshold; dangling weight loads
            # keep the clock gate open without contending for psum
            ldw_warms(20)
            def ldw_thunk(n):
                def t():
                    ldw_warms(n)
                return t

            attn_core(1, filler=q2 + [divide_finish(0, 0), divide_finish(0, 1)])
            q3 = qkv_pair_thunks(3)
            divide_start(1)
            ldw_warms(20)
            attn_core(2, filler=q3 + [divide_finish(1, 0), divide_finish(1, 1)])
            divide_start(2)
            ldw_warms(20)

            attn3_phaseA(filler=[divide_finish(2, 0), divide_finish(2, 1),
                                 ldw_thunk(6), ldw_thunk(6)]
                         + [proj_combo012(oi) for oi in range(4)])
            # ni=0 divide chain flies while phase B streams the ni=1 AVs
            div3_evac(0, 0)
            div3_evac(0, 1)
            div3_zq(0)
            div3_recip_rz(0)
            attn3_phaseB(range(6, 8))
            ldw_warms(8)
            div3_bcast(0)
            div3_mult(0)
            div3_evac(1, 0)
            div3_evac(1, 1)
            div3_zq(1)
            proj3_mm(0, 0, "s")
            proj3_fin(0, 0)
            proj3_mm(1, 0, "s")
            proj3_fin(1, 0)
            div3_recip_rz(1)
            proj3_mm(2, 0, "s")
            proj3_fin(2, 0)
            proj3_mm(3, 0, "s")
            proj3_fin(3, 0)
            ldw_warms(8)
            div3_bcast(1)
            div3_mult(1)
            proj3_mm(0, 1, "a")
            proj3_fin(0, 1)
            proj3_mm(1, 1, "a")
            proj3_fin(1, 1)
            store_pair(0)
            proj3_mm(2, 1, "a")
            proj3_fin(2, 1)
            proj3_mm(3, 1, "a")
            proj3_fin(3, 1)
            store_pair(1)

    _split_multi_waits(nc)
    return nc


def _prep_host(x, gamma, beta, w_qkv, b_qkv, w_proj, b_proj):
    """Host-side weight permutation/pre-scaling + per-core input maps."""
    import ml_dtypes
    bf16 = ml_dtypes.bfloat16
    # [b, c, t] -> [b, p, (ci, t)] so each partition's DMA read is contiguous
    x = np.ascontiguousarray(x, dtype=np.float32).reshape(B, 4, 128, T)
    xpack = x.transpose(0, 2, 1, 3).reshape(B, 128, 4096).astype(bf16)
    scale2 = 1.0 / np.sqrt(CH)  # folded into q (exact: 0.125 is a power of two)

    w = np.asarray(w_qkv, dtype=np.float32).reshape(H, 3, CH, C)
    bq = np.asarray(b_qkv, dtype=np.float32).reshape(H, 3, CH)
    wq = w[:, 0] * scale2          # [hd, 64, c]
    wk = w[:, 1]
    wv = w[:, 2]
    # per-pair packed [4, 128 c-part, (ci, q128|k128)]
    qblocks = wq.reshape(4, 128, C)     # pair pi rows: [q_{2pi}; q_{2pi+1}]
    kblocks = wk.reshape(4, 128, C)
    wqkP_host = np.zeros((4, 128, 4, 256), np.float32)
    for pi in range(4):
        pk = np.concatenate([qblocks[pi].T, kblocks[pi].T], axis=1)  # [512 c, 256]
        wqkP_host[pi] = pk.reshape(4, 128, 256).transpose(1, 0, 2)   # [p, ci, 256]
    # group pairs (0,1) and (2,3): [g, p, (pair, ci, 256)]
    wqkP_host = np.ascontiguousarray(
        wqkP_host.reshape(2, 2, 128, 1024).transpose(0, 2, 1, 3)
        .reshape(2, 128, 2048).astype(bf16))
    bqk_flat = np.concatenate(
        [(bq[:, 0] * scale2).reshape(4, 128), bq[:, 1].reshape(4, 128)], axis=0)  # [8, 128]
    wvT_host = np.ascontiguousarray(wv.transpose(2, 0, 1).reshape(C, C).astype(bf16))
    wpT_host = np.ascontiguousarray(np.asarray(w_proj, dtype=np.float32).T.astype(bf16))
    # v bias commutes through softmax (rows sum to 1): fold into proj bias
    bp_eff = np.asarray(b_proj, np.float32) + np.asarray(w_proj, np.float32) @ bq[:, 2].reshape(C)
    maskA_host = np.zeros((128, 8), np.float32)
    for p in range(128):
        maskA_host[p, p // 16] = 1.0
    maskB_host = np.ascontiguousarray(maskA_host.T)

    # packed consts [128, 20]: gam(4) bet(4) bqk(8, by oi) bp(4), each "(ci p)->p ci"
    cpack_host = np.zeros((128, 20), np.float32)
    cpack_host[:, 0:4] = np.asarray(gamma, np.float32).reshape(4, 128).T
    cpack_host[:, 4:8] = np.asarray(beta, np.float32).reshape(4, 128).T
    cpack_host[:, 8:16] = bqk_flat.T
    cpack_host[:, 16:20] = bp_eff.reshape(4, 128).T

    common = {
        "wqkP": wqkP_host, "wvT": wvT_host, "wpT": wpT_host,
        "cpack": np.ascontiguousarray(cpack_host),
        "maskA": maskA_host, "maskB": maskB_host,
    }
    return [dict(common, xin=np.ascontiguousarray(xpack[b])) for b in range(B)]


def kernel(x, gamma, beta, w_qkv, b_qkv, w_proj, b_proj):
    from concourse.bass_utils import run_bass_kernel_spmd

    if "nc" not in _CACHE:
        _CACHE["nc"] = _build_nc()
    nc = _CACHE["nc"]

    in_maps = _prep_host(x, gamma, beta, w_qkv, b_qkv, w_proj, b_proj)
    kwargs = {}
    if TRACE:
        _install_ntff_hook()
        kwargs["trace"] = True
    res = run_bass_kernel_spmd(nc, in_maps, core_ids=list(range(NCORES)), **kwargs)
    if TRACE:
        _CACHE["last_result"] = res
    # outd is [p, (ci, t)] packed; unpack to [c, t] and add the residual
    h = np.stack([np.asarray(r["outd"], np.float32) for r in res.results], axis=0)
    h = h.reshape(B, 128, 4, T).transpose(0, 2, 1, 3).reshape(B, C, T)
    out = np.asarray(x, np.float32).reshape(B, C, T) + h
    return out.reshape(B, C, HW, HW)



# revision 32
# speedup vs baseline: 1.0253x; 1.0253x over previous
"""Trainium2 Bass kernel for nn_AttentionBlock (GroupNorm + MHA + proj + residual).

Sharding: data-parallel over batch; 8 batches -> 8 NeuronCores, one batch each.

v2 design (vs baseline at 181us):
  - bf16 on the whole matmul path (x, weights, h, q, k, es, vTa, at); GN stats,
    psum accumulation, softmax Z and the residual stay f32.  Halves input DMA.
  - input DMA spread over 4 engine queues, x first; consts packed into one DMA.
  - warmup matmuls on a scratch tile bridge the HAM activity window so real
    matmuls run at 2.4 GHz from the start.
  - score matmuls K=64 per head, packed as concurrent PE row-tiles
    (head even on partitions 0-63, head odd on 64-127).
  - softmax denominator via ones-column in vTa (M=65 AV matmuls); normalization
    via DVE reciprocal of the psum Z row + gpsimd partition_broadcast + one
    fused DVE multiply.  No DRAM round trips.
  - output stores split per 512-col half across all 4 queues.
"""

import numpy as np

B, C, HW, T = 8, 512, 32, 1024
H, CH = 8, 64
G, GS = 32, 16
EPS = 1e-5
NCORES = 8

WARM_N = 28       # warmup matmuls (N=512) to hold the HAM window until real work
USE_PB = False    # gpsimd partition_broadcast for 1/Z (unsupported by this
                  # walrus: "ISA wrong length"); else PE broadcast

_CACHE = {}
TRACE = False  # test harness can set kernel.TRACE = True to get a profile


def _install_ntff_hook():
    import sys, types
    if 'antenv.axon_hooks' in sys.modules:
        return
    mod = types.ModuleType('antenv.axon_hooks')
    state = {'hook': None}
    mod.set_axon_ntff_profile_hook = lambda h: state.__setitem__('hook', h)
    mod.get_axon_ntff_profile_hook = lambda: state['hook']
    sys.modules['antenv.axon_hooks'] = mod
    import antenv
    antenv.axon_hooks = mod
    try:
        from trn_agent_boot.trn_boot import _ntff_profile_via_ctypes
        mod.set_axon_ntff_profile_hook(_ntff_profile_via_ctypes('/opt/axon/libaxon_pjrt.so'))
    except Exception:
        pass


def _patch_ldw_opt():
    """Let walrus dedup back-to-back LDWEIGHTS of the same stationary operand."""
    import concourse.bass_utils as bu
    if getattr(bu, "_ldw_opt_patched", False):
        return
    orig = bu.run_command

    def patched(argv, **kw):
        argv = ["--enable-ldw-opt=true" if a == "--enable-ldw-opt=false" else a
                for a in argv]
        return orig(argv, **kw)

    bu.run_command = patched
    bu._ldw_opt_patched = True


def _split_multi_waits(nc, max_waits=1):
    """This container's walrus supports only one sync wait per instruction; move
    extra waits onto same-engine no-ops inserted just before the instruction."""
    import concourse.mybir as mybir
    for f in nc.m.functions:
        for bb in f.blocks:
            insts = bb.instructions
            out = []
            changed = False
            for inst in insts:
                si = inst.sync_info
                waits = list(si.on_wait) if si is not None and si.on_wait else []
                if len(waits) > max_waits:
                    changed = True
                    for j, w in enumerate(waits[:-max_waits]):
                        out.append(mybir.InstNoOp(
                            name=f"{inst.name}-ws{j}",
                            sync_info=mybir.SyncInfo(on_wait=[w], on_update=[]),
                            bass_nofuse=True,
                            engine=inst.engine,
                        ))
                    inst.sync_info = mybir.SyncInfo(
                        on_wait=waits[-max_waits:],
                        on_update=list(si.on_update) if si.on_update else [],
                    )
                out.append(inst)
            if changed:
                bb.instructions = out


def _build_nc():
    import concourse.bass as bass
    import concourse.tile as tile
    import concourse.mybir as mybir

    f32 = mybir.dt.float32
    f32r = mybir.dt.float32r
    bf16 = mybir.dt.bfloat16
    Alu = mybir.AluOpType
    Act = mybir.ActivationFunctionType

    nc = bass.Bass()

    # x packed host-side as [p, (ci, t)] so each partition reads 4KB+
    # contiguous (2KB descriptors halve the effective DMA rate)
    xin = nc.dram_tensor("xin", [128, 4096], bf16, kind="ExternalInput")
    # q|k weights packed by pair-group: [g, c-part, (pair, ci, q128|k128)]
    wqkP = nc.dram_tensor("wqkP", [2, 128, 2048], bf16, kind="ExternalInput")
    wvT = nc.dram_tensor("wvT", [C, C], bf16, kind="ExternalInput")
    wpT = nc.dram_tensor("wpT", [C, C], bf16, kind="ExternalInput")
    # packed small consts: cols 0:4 gam, 4:8 bet, 8:16 bqk(by oi), 16:20 bp
    cpack = nc.dram_tensor("cpack", [128, 20], f32, kind="ExternalInput")
    maskA = nc.dram_tensor("maskA", [128, 8], f32r, kind="ExternalInput")
    maskB = nc.dram_tensor("maskB", [8, 128], f32r, kind="ExternalInput")
    # h only, [p, (ci, t)] packed bf16; residual x + unpack happen host-side
    outd = nc.dram_tensor("outd", [128, 4096], bf16, kind="ExternalOutput")

    store_engs = []  # round-robin for output stores

    with tile.TileContext(nc) as tc:
        with tc.tile_pool(name="const", bufs=1) as const, \
             tc.tile_pool(name="big", bufs=1) as big, \
             tc.tile_pool(name="qpp", bufs=2) as qpp, \
             tc.tile_pool(name="kpp", bufs=2) as kpp, \
             tc.tile_pool(name="esp", bufs=16) as esp, \
             tc.tile_pool(name="accp", bufs=1) as accp, \
             tc.tile_pool(name="zp", bufs=2) as zp, \
             tc.tile_pool(name="gn", bufs=2) as gn, \
             tc.tile_pool(name="ps", bufs=2, space="PSUM") as ps, \
             tc.tile_pool(name="dram", bufs=2, space="DRAM") as dram:

            # ---- warmup scaffolding; actual warm matmuls are emitted in
            # phases below (plain first, then gated on x arrivals) so the PE
            # stays busy through the DMA/GroupNorm window without blocking
            # real work.  warm memset on gpsimd so the tensor queue can start
            # ~2us before the DVE preamble finishes. ----
            warm = const.tile([128, 512], bf16)
            nc.gpsimd.memset(warm.bitcast(f32), 0.0)
            wi = [0]

            def warms(n, src=None):
                src = warm if src is None else src
                for _ in range(n):
                    pw = ps.tile([128, 512], f32, tag="s", name=f"warm{wi[0]}")
                    nc.tensor.matmul(pw, lhsT=src[:, 0:128],
                                     rhs=src[:, 0:512], start=True, stop=True)
                    wi[0] += 1

            # one contiguous burst > the 3.4us HAM window so the clock gate
            # actually opens; the x-gated batches below then keep it open.
            warms(10)
            onesr = const.tile([1, 64], f32r)
            nc.vector.memset(onesr.bitcast(f32), 1.0)
            ones1p = const.tile([1, 128], f32r)
            nc.vector.memset(ones1p.bitcast(f32), 1.0)
            onesf = const.tile([128, 64], f32)
            nc.vector.memset(onesf, 1.0)

            # ---- input DMA.  Only sync/scalar/gpsimd queues can issue DMAs;
            # the gpsimd (software-DGE) queue is ~4x slower, so it only gets
            # wpT (needed last).  x first in 4 chunks so GroupNorm stats can
            # start per-chunk; weights strictly behind x on both fast rings.
            xt2 = [big.tile([128, 2, 1024], bf16, tag=f"x{g}", name=f"xg{g}")
                   for g in range(2)]
            xtv = [xt2[ci // 2][:, ci % 2, :] for ci in range(4)]
            # tiny consts on the (otherwise idle) gpsimd ring so they don't
            # delay the scalar ring's x chunk
            cp_t = const.tile([128, 20], f32)
            nc.gpsimd.dma_start(out=cp_t, in_=cpack[:, :])
            mA = const.tile([128, 8], f32r)
            nc.gpsimd.dma_start(out=mA, in_=maskA[:, :])
            mB = const.tile([8, 128], f32r)
            nc.gpsimd.dma_start(out=mB, in_=maskB[:, :])
            xinr = xin.rearrange("p (ci t) -> p ci t", t=1024)
            nc.sync.dma_start(out=xt2[0][:, 0, :], in_=xinr[:, 0, :])
            nc.sync.dma_start(out=xt2[0][:, 1, :], in_=xinr[:, 1, :])
            nc.scalar.dma_start(out=xt2[1][:, 0, :], in_=xinr[:, 2, :])
            nc.scalar.dma_start(out=xt2[1][:, 1, :], in_=xinr[:, 3, :])
            gam_t = cp_t[:, 0:4]
            bet_t = cp_t[:, 4:8]
            bqk_t = cp_t[:, 8:16]
            bp_t = cp_t[:, 16:20]

            # weights behind x: qk pairs 0/1 on sync, wvT then qk pairs 2/3
            # on scalar, wpT (needed last) on the slow gpsimd queue
            wq2 = [const.tile([128, 2, 4, 256], bf16, tag=f"wqg{g}", name=f"wqg{g}")
                   for g in range(2)]
            wq_t = [wq2[pi // 2][:, pi % 2] for pi in range(4)]
            wvT_t = const.tile([128, 4, 512], bf16)
            nc.sync.dma_start(out=wq2[0], in_=wqkP[0])
            nc.scalar.dma_start(out=wvT_t, in_=wvT.rearrange("(ci p) o -> p ci o", p=128))
            nc.scalar.dma_start(out=wq2[1], in_=wqkP[1])
            wpT_t = const.tile([128, 4, 512], bf16)
            nc.gpsimd.dma_start(out=wpT_t, in_=wpT.rearrange("(ci p) o -> p ci o", p=128))

            # dummy exp after the doorbells so the ACT table set loads during
            # the DMA wait without delaying the scalar ring's x chunk
            dummy = const.tile([1, 4], f32)
            nc.scalar.activation(out=dummy, in_=warm.bitcast(f32)[0:1, 0:4],
                                 func=Act.Exp)
            def ldw_warms(n):
                # keep the PE array streaming without touching psum or the
                # tile pools: dangling weight loads only
                for _ in range(n):
                    nc.tensor.ldweights(weights=warm[:, 0:128])

            # keep the PE warm while x lands; the GN matmuls (pg, pc) are
            # interleaved below so they run as soon as their DVE inputs land
            warms(2, src=xtv[0])
            warms(2, src=xtv[1])

            # ---- GroupNorm ----
            chmv = gn.tile([128, 4, 2], f32)
            for ci in range(4):
                st = gn.tile([128, 2, 6], f32, tag="st")
                xv = xtv[ci].rearrange("p (n f) -> p n f", f=512)
                for sub in range(2):
                    nc.vector.bn_stats(out=st[:, sub, :], in_=xv[:, sub, :])
                nc.vector.bn_aggr(out=chmv[:, ci, :], in_=st)
            s2ch = gn.tile([128, 4, 2], f32r)
            nc.vector.tensor_copy(out=s2ch[:, :, 0], in_=chmv[:, :, 0])
            t1 = gn.tile([128, 4], f32)
            nc.vector.tensor_mul(out=t1, in0=chmv[:, :, 0], in1=chmv[:, :, 0])
            nc.vector.tensor_add(out=s2ch[:, :, 1], in0=t1, in1=chmv[:, :, 1])
            pg = ps.tile([128, 1024], f32, tag="s", name="pgn")
            nc.tensor.matmul(pg[0:8, 0:8], lhsT=mA,
                             rhs=s2ch.rearrange("p a b -> p (a b)"),
                             start=True, stop=True)
            # bridge the DVE Newton window so the PE clock stays up between
            # the pg and pc matmuls
            warms(6, src=xtv[2])
            warms(6, src=xtv[3])
            gf = gn.tile([8, 4, 2], f32r)
            mg = gn.tile([8, 4], f32)
            nc.vector.tensor_scalar_mul(out=mg, in0=pg[0:8, 0:8].rearrange(
                "g (a b) -> g a b", b=2)[:, :, 0], scalar1=1.0 / GS)
            vg = gn.tile([8, 4], f32)
            nc.vector.tensor_scalar_mul(out=vg, in0=pg[0:8, 0:8].rearrange(
                "g (a b) -> g a b", b=2)[:, :, 1], scalar1=1.0 / GS)
            m2 = gn.tile([8, 4], f32)
            nc.vector.tensor_mul(out=m2, in0=mg, in1=mg)
            nc.vector.tensor_sub(out=vg, in0=vg, in1=m2)
            # rstd = 1/sqrt(v+eps) via Newton on DVE (v is ~1 for unit-normal
            # x so x0 = 1.5-0.5v has rel err <6e-3; one iteration squares it
            # to <5e-5 which is far below the bf16 noise floor)
            nc.vector.tensor_scalar_add(out=vg, in0=vg, scalar1=EPS)
            yv = gn.tile([8, 4], f32)
            nc.vector.tensor_scalar(out=yv, in0=vg, scalar1=-0.5, scalar2=1.5,
                                    op0=Alu.mult, op1=Alu.add)
            for _ in range(1):
                yy = gn.tile([8, 4], f32, tag="yy")
                nc.vector.tensor_mul(out=yy, in0=yv, in1=yv)
                nc.vector.tensor_mul(out=yy, in0=yy, in1=vg)
                nc.vector.tensor_scalar(out=yy, in0=yy, scalar1=-0.5, scalar2=1.5,
                                        op0=Alu.mult, op1=Alu.add)
                nc.vector.tensor_mul(out=yv, in0=yv, in1=yy)
            nc.vector.tensor_copy(out=gf[:, :, 0], in_=mg)
            nc.vector.tensor_copy(out=gf[:, :, 1], in_=yv)
            pc = ps.tile([128, 1024], f32, tag="s", name="pgc")
            nc.tensor.matmul(pc[:, 0:8], lhsT=mB,
                             rhs=gf.rearrange("g a b -> g (a b)"),
                             start=True, stop=True)
            chms = pc[:, 0:8].rearrange("p (a b) -> p a b", b=2)
            scl = gn.tile([128, 4], f32)
            nc.vector.tensor_mul(out=scl, in0=gam_t, in1=chms[:, :, 1])
            sht = gn.tile([128, 4], f32)
            nc.vector.tensor_mul(out=sht, in0=scl, in1=chms[:, :, 0])
            nc.vector.tensor_sub(out=sht, in0=bet_t, in1=sht)
            ldw_warms(6)
            # h = x * scl + sht  (bf16; DVE for ci 0,2; ACT (idle until the
            # first softmax) for ci 1,3 so qkv isn't gated on one engine)
            ht = [big.tile([128, 1024], bf16, tag=f"h{ci}", name=f"h{ci}") for ci in range(4)]
            for ci in range(4):
                if ci % 2 == 0:
                    nc.vector.tensor_scalar(out=ht[ci], in0=xtv[ci],
                                            scalar1=scl[:, ci:ci + 1], scalar2=sht[:, ci:ci + 1],
                                            op0=Alu.mult, op1=Alu.add)
                else:
                    nc.scalar.activation(out=ht[ci], in_=xtv[ci], func=Act.Identity,
                                         scale=scl[:, ci:ci + 1], bias=sht[:, ci:ci + 1])

            # ---- vT[s, (ti, hd, ch+1)]; ones column memset once up front ----
            vTa = big.tile([128, 8, 8, 65], bf16)
            nc.vector.tensor_copy(
                out=vTa[:, :, :, 64:65],
                in_=onesf.rearrange("p (a b c) -> p a b c", a=8, b=8))

            def v_thunk(ti):
                # bv is folded into bp host-side (softmax rows sum to 1), so
                # the evacuation is a plain copy.  Per-ti granularity keeps
                # filler chunks small so the exp stream isn't starved.
                def t():
                    pv = ps.tile([128, 512], f32, tag="s", name=f"pv{ti}")
                    for ci in range(4):
                        nc.tensor.matmul(pv,
                                         lhsT=ht[ci][:, ti * 128:(ti + 1) * 128],
                                         rhs=wvT_t[:, ci, :], start=(ci == 0), stop=(ci == 3))
                    nc.vector.tensor_copy(
                        out=vTa[:, ti, :, 0:64],
                        in_=pv.rearrange("p (h c) -> p h c", h=8))
                return t

            # ---- q,k for one head pair.  qp: [q_even; q_odd] on 128 partitions,
            # kp: [k_even; k_odd] likewise (no zero padding: scores use K=64
            # row-tiles so the two heads run concurrently on the PE). ----
            qp = [None] * 4
            kp = [None] * 4

            def qkv_pair_thunks(pi):
                qp[pi] = qpp.tile([128, 1024], bf16, tag="qp", name=f"qp{pi}")
                kp[pi] = kpp.tile([128, 1024], bf16, tag="kp", name=f"kp{pi}")
                thunks = []
                state = {}

                def mk_mm(side, ci):
                    def t():
                        oi = side * 4 + pi
                        if ci == 0:
                            state[side] = ps.tile([128, 1024], f32, tag="s",
                                                  name=f"pqk{oi}")
                        pqk = state[side]
                        for ni in range(2):
                            nc.tensor.matmul(pqk[:, ni * 512:(ni + 1) * 512],
                                             lhsT=wq_t[pi][:, ci, side * 128:(side + 1) * 128],
                                             rhs=ht[ci][:, ni * 512:(ni + 1) * 512],
                                             start=(ci == 0), stop=(ci == 3))
                    return t

                def mk_evac(side):
                    def t():
                        oi = side * 4 + pi
                        dst = qp[pi] if side == 0 else kp[pi]
                        nc.vector.tensor_scalar_add(out=dst, in0=state[side],
                                                    scalar1=bqk_t[:, oi:oi + 1])
                    return t

                for side in range(2):
                    for ci in range(4):
                        thunks.append(mk_mm(side, ci))
                    thunks.append(mk_evac(side))
                return thunks

            # ---- attention core (one head pair), with PE row-tile packing on
            # the score matmuls and filler thunks to fill PE slack ----
            at_ = [None] * 4
            acc_all = accp.tile([128, 4, 1024], bf16)
            acc = [acc_all[:, oi, :] for oi in range(4)]
            pa_all = [None] * 4

            def attn_core(pi, filler=()):
                filler = list(filler)
                pa = [ps.tile([128, 1024], f32, tag="a", name=f"pa{pi}_{i}") for i in range(2)]
                pa_all[pi] = pa
                for si in range(8):
                    if si >= 1 and filler:
                        # pace fillers so none spill past the si loop (spills
                        # land on the pair transition and stall the exp
                        # stream long enough to re-throttle the PE clock)
                        n = -(-len(filler) // (8 - si))
                        for _ in range(n):
                            if filler:
                                filler.pop(0)()
                    pss = [ps.tile([128, 1024], f32, tag="s", name=f"pss{pi}_{si}_{i}")
                           for i in range(2)]
                    # concurrent row-tiles: head even rows 0-63, head odd 64-127
                    for ni in range(2):
                        for half in range(2):
                            lo, hi = half * 64, half * 64 + 64
                            nc.tensor.matmul(
                                pss[half][:, ni * 512:(ni + 1) * 512],
                                lhsT=kp[pi][lo:hi, si * 128:(si + 1) * 128],
                                rhs=qp[pi][lo:hi, ni * 512:(ni + 1) * 512],
                                start=True, stop=True)
                    ess = []
                    for half in range(2):
                        es = esp.tile([128, 1024], bf16, tag="es")
                        nc.scalar.activation(out=es, in_=pss[half], func=Act.Exp)
                        ess.append(es)
                    for half in range(2):
                        hd = 2 * pi + half
                        for ni in range(2):
                            nc.tensor.matmul(pa[half][0:65, ni * 512:(ni + 1) * 512],
                                             lhsT=vTa[:, si, hd, :],
                                             rhs=ess[half][:, ni * 512:(ni + 1) * 512],
                                             start=(si == 0), stop=(si == 7))
                for t in filler:
                    t()

            div_state = [None] * 4

            def divide_start(pi, act_copy=False):
                # Z reciprocal via [128, 8] reshape: one sbuf->sbuf reshape
                # DMA, reciprocal, then a DRAM bounce for the broadcast read.
                # Evacuate pa first so its psum banks free immediately; for
                # the last pair half 1 evacuates on the (now idle) ACT engine
                # so both evacs run concurrently.
                at_[pi] = big.tile([128, 1024], bf16, tag=f"at{pi}", name=f"at{pi}")
                pa = pa_all[pi]
                aus, zbs = [], []
                for half in range(2):
                    au = zp.tile([65, 1024], f32, tag="au")
                    if act_copy and half == 1:
                        nc.scalar.activation(out=au, in_=pa[half][0:65, :],
                                             func=Act.Copy)
                    else:
                        nc.vector.tensor_copy(out=au, in_=pa[half][0:65, :])
                    aus.append(au)
                for half in range(2):
                    eng = nc.sync if half == 0 else nc.scalar
                    zq = zp.tile([128, 8], f32, tag="zq")
                    eng.dma_start(out=zq, in_=aus[half][64:65, :].rearrange(
                        "p (a b) -> p a b", b=8))
                    nc.vector.reciprocal(out=zq, in_=zq)
                    zd2 = dram.tile([1, 1024], f32, tag="zd2")
                    eng.dma_start(
                        out=bass.AP(tensor=zd2.tensor, offset=zd2.offset,
                                    ap=[[8, 128], [1, 8]]), in_=zq)
                    zb = zp.tile([64, 1024], f32, tag="zb")
                    eng.dma_start(out=zb, in_=bass.AP(
                        tensor=zd2.tensor, offset=zd2.offset, ap=[[0, 64], [1, 1024]]))
                    zbs.append(zb)
                div_state[pi] = (aus, zbs)

            def divide_finish(pi, half):
                # emitted well after divide_start so the in-order DVE queue
                # never stalls waiting for the bounce
                def t():
                    aus, zbs = div_state[pi]
                    nc.vector.tensor_tensor(
                        out=at_[pi][half * 64:half * 64 + 64, :],
                        in0=aus[half][0:64, :], in1=zbs[half], op=Alu.mult)
                return t

            def proj_combo012(oi):
                # pi 0..2 contributions accumulate in psum; one fused evac
                # adds the bias (residual x is added host-side).  Runs as
                # phase-A fillers so only pair 3's proj piece is in the tail.
                def t():
                    pp = ps.tile([128, 1024], f32, tag="s", name=f"pp012_{oi}")
                    for pi in range(3):
                        for ni in range(2):
                            nc.tensor.matmul(pp[:, ni * 512:(ni + 1) * 512],
                                             lhsT=wpT_t[:, pi, oi * 128:(oi + 1) * 128],
                                             rhs=at_[pi][:, ni * 512:(ni + 1) * 512],
                                             start=(pi == 0), stop=(pi == 2))
                    nc.vector.tensor_scalar_add(out=acc[oi], in0=pp,
                                                scalar1=bp_t[:, oi:oi + 1])
                return t

            # ---- pair 3: ni-split attention so Z(t) for the first t-half is
            # final while the second half's AV still streams; softmax divide
            # via DVE recip on a [128,4] reshape + PE row-broadcast of the
            # reciprocal into pa's unused rows 64:127.  No DRAM round trip.
            es3 = []

            def attn3_phaseA(filler=()):
                filler = list(filler)
                pa = [ps.tile([128, 1024], f32, tag="a", name=f"pa3_{i}") for i in range(2)]
                pa_all[3] = pa
                for si in range(8):
                    if si >= 1 and filler:
                        n = -(-len(filler) // (8 - si))
                        for _ in range(n):
                            if filler:
                                filler.pop(0)()
                    pss = [ps.tile([128, 1024], f32, tag="s", name=f"pss3_{si}_{i}")
                           for i in range(2)]
                    for ni in range(2):
                        for half in range(2):
                            lo, hi = half * 64, half * 64 + 64
                            nc.tensor.matmul(
                                pss[half][:, ni * 512:(ni + 1) * 512],
                                lhsT=kp[3][lo:hi, si * 128:(si + 1) * 128],
                                rhs=qp[3][lo:hi, ni * 512:(ni + 1) * 512],
                                start=True, stop=True)
                    # ni=1 AVs lag 2 si behind: keeps the PE duty high enough
                    # through phase A that the HAM never drops the clock, and
                    # shrinks phase B to the last two si.  Emitted BEFORE this
                    # si's exp-gated AVs so they fill the exp wait.
                    if si >= 2:
                        sj = si - 2
                        for half in range(2):
                            nc.tensor.matmul(pa[half][0:65, 512:1024],
                                             lhsT=vTa[:, sj, 6 + half, :],
                                             rhs=es3[sj][half][:, 512:1024],
                                             start=(sj == 0), stop=False)
                    ess = []
                    for half in range(2):
                        es = esp.tile([128, 1024], bf16, tag="es")
                        nc.scalar.activation(out=es, in_=pss[half], func=Act.Exp)
                        ess.append(es)
                    es3.append(ess)
                    for half in range(2):
                        nc.tensor.matmul(pa[half][0:65, 0:512],
                                         lhsT=vTa[:, si, 6 + half, :],
                                         rhs=ess[half][:, 0:512],
                                         start=(si == 0), stop=(si == 7))
                for t in filler:
                    t()

            def attn3_phaseB(si_range):
                pa = pa_all[3]
                for si in si_range:
                    for half in range(2):
                        nc.tensor.matmul(pa[half][0:65, 512:1024],
                                         lhsT=vTa[:, si, 6 + half, :],
                                         rhs=es3[si][half][:, 512:1024],
                                         start=False, stop=(si == 7))

            at3 = big.tile([128, 1024], bf16, name="at3")
            au3 = {}
            rz3 = {}

            def div3_evac(ni, half):
                # numerator (and the Z row in row 64) psum -> sbuf; half 1 on
                # the ACT engine which is idle after the last exp.  One tile
                # per (ni, half) so the framework never invents cross-ni deps.
                au = zp.tile([65, 512], f32, tag="au3", bufs=4,
                             name=f"au3_{ni}_{half}")
                au3[(ni, half)] = au
                src = pa_all[3][half][0:65, ni * 512:(ni + 1) * 512]
                if half == 1:
                    nc.scalar.activation(out=au, in_=src, func=Act.Copy)
                else:
                    nc.vector.tensor_copy(out=au, in_=src)

            def div3_zq(ni):
                for half in range(2):
                    eng = nc.sync if half == 0 else nc.scalar
                    zq = zp.tile([64, 8], f32, tag="zq3", name=f"zq3_{ni}_{half}")
                    eng.dma_start(out=zq, in_=au3[(ni, half)][64:65, :]
                                  .rearrange("p (a b) -> p a b", b=8))
                    rz3[(ni, half, "zq")] = zq

            def div3_recip_rz(ni):
                for half in range(2):
                    eng = nc.sync if half == 0 else nc.scalar
                    zq = rz3[(ni, half, "zq")]
                    nc.vector.reciprocal(out=zq, in_=zq)
                    rz = zp.tile([1, 512], f32r, tag="rz3", name=f"rz3_{ni}_{half}")
                    eng.dma_start(out=rz.bitcast(f32).rearrange(
                        "p (a b) -> p a b", b=8), in_=zq)
                    rz3[(ni, half)] = rz

            def div3_bcast(ni):
                # 1/Z row -> all 128 pa rows via a K=1 ones matmul (matmul
                # dst must start at partition 0); the numerator was already
                # evacuated to au3 so the overwrite is safe
                for half in range(2):
                    nc.tensor.matmul(
                        pa_all[3][half][:, ni * 512:(ni + 1) * 512],
                        lhsT=ones1p, rhs=rz3[(ni, half)], start=True, stop=True)

            def div3_mult(ni):
                for half in range(2):
                    nc.vector.tensor_tensor(
                        out=at3[half * 64:half * 64 + 64, ni * 512:(ni + 1) * 512],
                        in0=au3[(ni, half)][0:64, :],
                        in1=pa_all[3][half][0:64, ni * 512:(ni + 1) * 512],
                        op=Alu.mult)

            pp3 = {}

            def proj3_mm(oi, ni, tag):
                pp = ps.tile([128, 512], f32, tag=tag, name=f"pp3_{oi}_{ni}")
                pp3[(oi, ni)] = pp
                nc.tensor.matmul(pp, lhsT=wpT_t[:, 3, oi * 128:(oi + 1) * 128],
                                 rhs=at3[:, ni * 512:(ni + 1) * 512],
                                 start=True, stop=True)

            def proj3_fin(oi, ni):
                nc.vector.tensor_add(
                    out=acc_all[:, oi, ni * 512:(ni + 1) * 512],
                    in0=acc_all[:, oi, ni * 512:(ni + 1) * 512],
                    in1=pp3[(oi, ni)])

            def store_pair(opair):
                # split by partition halves across both fast rings: full-rate
                # 4KB descriptors, half the bytes per queue
                outr = outd.rearrange("p (ci t) -> p ci t", t=1024)
                lo, hi = opair * 2, opair * 2 + 2
                nc.sync.dma_start(out=outr[0:64, lo:hi, :],
                                  in_=acc_all[0:64, lo:hi, :])
                nc.scalar.dma_start(out=outr[64:128, lo:hi, :],
                                    in_=acc_all[64:128, lo:hi, :])

            # ---- schedule ----
            q0 = qkv_pair_thunks(0)
            for t in q0:
                t()
            v_thunks = [v_thunk(ti) for ti in range(8)]
            v_thunks[0]()
            q1 = qkv_pair_thunks(1)
            # interleave so v(ti) stays ahead of AV(si)
            f0 = [v_thunks[1], v_thunks[2], v_thunks[3], q1[0],
                  v_thunks[4], q1[1], v_thunks[5], q1[2],
                  v_thunks[6], q1[3], v_thunks[7]] + q1[4:]
            attn_core(0, filler=f0)
            q2 = qkv_pair_thunks(2)
            divide_start(0)
            # transition warms: pair hand-offs idle the PE for ~2-3.4us,
            # right at the HAM re-throttle threshold; dangling weight loads
            # keep the clock gate open without contending for psum
            ldw_warms(20)
            def ldw_thunk(n):
                def t():
                    ldw_warms(n)
                return t

            attn_core(1, filler=q2 + [divide_finish(0, 0), divide_finish(0, 1)])
            q3 = qkv_pair_thunks(3)
            divide_start(1)
            ldw_warms(20)
            attn_core(2, filler=q3 + [divide_finish(1, 0), divide_finish(1, 1)])
            divide_start(2)
            ldw_warms(20)

            attn3_phaseA(filler=[divide_finish(2, 0), divide_finish(2, 1),
                                 ldw_thunk(6), ldw_thunk(6)]
                         + [proj_combo012(oi) for oi in range(4)])
            # ni=0 divide chain flies while phase B streams the ni=1 AVs
            div3_evac(0, 0)
            div3_evac(0, 1)
            div3_zq(0)
            div3_recip_rz(0)
            attn3_phaseB(range(6, 8))
            div3_evac(1, 0)
            div3_evac(1, 1)
            div3_zq(1)
            warms(8)
            div3_bcast(0)
            div3_mult(0)
            div3_recip_rz(1)
            proj3_mm(0, 0, "s")
            proj3_fin(0, 0)
            proj3_mm(1, 0, "s")
            proj3_fin(1, 0)
            proj3_mm(2, 0, "s")
            proj3_fin(2, 0)
            proj3_mm(3, 0, "s")
            proj3_fin(3, 0)
            warms(4)
            ldw_warms(4)
            div3_bcast(1)
            div3_mult(1)
            proj3_mm(0, 1, "a")
            proj3_fin(0, 1)
            proj3_mm(1, 1, "a")
            proj3_fin(1, 1)
            store_pair(0)
            proj3_mm(2, 1, "a")
            proj3_fin(2, 1)
            proj3_mm(3, 1, "a")
            proj3_fin(3, 1)
            store_pair(1)

    _split_multi_waits(nc)
    return nc


def _prep_host(x, gamma, beta, w_qkv, b_qkv, w_proj, b_proj):
    """Host-side weight permutation/pre-scaling + per-core input maps."""
    import ml_dtypes
    bf16 = ml_dtypes.bfloat16
    # [b, c, t] -> [b, p, (ci, t)] so each partition's DMA read is contiguous
    x = np.ascontiguousarray(x, dtype=np.float32).reshape(B, 4, 128, T)
    xpack = x.transpose(0, 2, 1, 3).reshape(B, 128, 4096).astype(bf16)
    scale2 = 1.0 / np.sqrt(CH)  # folded into q (exact: 0.125 is a power of two)

    w = np.asarray(w_qkv, dtype=np.float32).reshape(H, 3, CH, C)
    bq = np.asarray(b_qkv, dtype=np.float32).reshape(H, 3, CH)
    wq = w[:, 0] * scale2          # [hd, 64, c]
    wk = w[:, 1]
    wv = w[:, 2]
    # per-pair packed [4, 128 c-part, (ci, q128|k128)]
    qblocks = wq.reshape(4, 128, C)     # pair pi rows: [q_{2pi}; q_{2pi+1}]
    kblocks = wk.reshape(4, 128, C)
    wqkP_host = np.zeros((4, 128, 4, 256), np.float32)
    for pi in range(4):
        pk = np.concatenate([qblocks[pi].T, kblocks[pi].T], axis=1)  # [512 c, 256]
        wqkP_host[pi] = pk.reshape(4, 128, 256).transpose(1, 0, 2)   # [p, ci, 256]
    # group pairs (0,1) and (2,3): [g, p, (pair, ci, 256)]
    wqkP_host = np.ascontiguousarray(
        wqkP_host.reshape(2, 2, 128, 1024).transpose(0, 2, 1, 3)
        .reshape(2, 128, 2048).astype(bf16))
    bqk_flat = np.concatenate(
        [(bq[:, 0] * scale2).reshape(4, 128), bq[:, 1].reshape(4, 128)], axis=0)  # [8, 128]
    wvT_host = np.ascontiguousarray(wv.transpose(2, 0, 1).reshape(C, C).astype(bf16))
    wpT_host = np.ascontiguousarray(np.asarray(w_proj, dtype=np.float32).T.astype(bf16))
    # v bias commutes through softmax (rows sum to 1): fold into proj bias
    bp_eff = np.asarray(b_proj, np.float32) + np.asarray(w_proj, np.float32) @ bq[:, 2].reshape(C)
    maskA_host = np.zeros((128, 8), np.float32)
    for p in range(128):
        maskA_host[p, p // 16] = 1.0
    maskB_host = np.ascontiguousarray(maskA_host.T)

    # packed consts [128, 20]: gam(4) bet(4) bqk(8, by oi) bp(4), each "(ci p)->p ci"
    cpack_host = np.zeros((128, 20), np.float32)
    cpack_host[:, 0:4] = np.asarray(gamma, np.float32).reshape(4, 128).T
    cpack_host[:, 4:8] = np.asarray(beta, np.float32).reshape(4, 128).T
    cpack_host[:, 8:16] = bqk_flat.T
    cpack_host[:, 16:20] = bp_eff.reshape(4, 128).T

    common = {
        "wqkP": wqkP_host, "wvT": wvT_host, "wpT": wpT_host,
        "cpack": np.ascontiguousarray(cpack_host),
        "maskA": maskA_host, "maskB": maskB_host,
    }
    return [dict(common, xin=np.ascontiguousarray(xpack[b])) for b in range(B)]


def kernel(x, gamma, beta, w_qkv, b_qkv, w_proj, b_proj):
    from concourse.bass_utils import run_bass_kernel_spmd

    if "nc" not in _CACHE:
        _CACHE["nc"] = _build_nc()
    nc = _CACHE["nc"]

    in_maps = _prep_host(x, gamma, beta, w_qkv, b_qkv, w_proj, b_proj)
    kwargs = {}
    if TRACE:
        _install_ntff_hook()
        kwargs["trace"] = True
    res = run_bass_kernel_spmd(nc, in_maps, core_ids=list(range(NCORES)), **kwargs)
    if TRACE:
        _CACHE["last_result"] = res
    # outd is [p, (ci, t)] packed; unpack to [c, t] and add the residual
    h = np.stack([np.asarray(r["outd"], np.float32) for r in res.results], axis=0)
    h = h.reshape(B, 128, 4, T).transpose(0, 2, 1, 3).reshape(B, C, T)
    out = np.asarray(x, np.float32).reshape(B, C, T) + h
    return out.reshape(B, C, HW, HW)

